# revision 2
# baseline (speedup 1.0000x reference)
"""Bass/Trainium2 kernel for nn_BaseAttention (B=2, N=2048, D=1024, H=16 causal).

Sharding: batch x head-group. Core c handles batch c//4 and heads
[4*(c%4), 4*(c%4)+4) (feature slice of 256 = 2 head-pairs). Each core
computes q/k/v projections for its slice from its batch's (pre-transposed)
x, runs causal attention for its 4 heads, applies its slice of the output
projection (row-parallel Wo), and writes a bf16 [2048, 1024] partial that
the host sums per batch.

Device dataflow (per core, matmuls bf16 -> fp32 PSUM):
  xt (SBUF)  --matmul-->  qT, kT   [2 pair-chunks x 128 feats, 2048 toks]
             --matmul-->  V directly in [tok, feat] layout (xt tile is the
                          stationary operand), stored per 128-tok k-tile as
                          4 x [V_h (64) | ones] slots for the EV matmuls
  For each (q-chunk, pair, k-tile):
    S^T[k, q] for BOTH heads of the pair in one PSUM tile [128, 1024]:
      head0 uses PE row group 0-1 (contraction partitions 0:64),
      head1 row group 2-3 (partitions 64:128) -- the two K=64 matmuls run
      concurrently via tile_position auto-derivation, costing ~1x N cycles.
    E^T = exp(S^T) on ACT (no max subtraction: |logits| < ~4 for this data)
    [O_h; den_h] = [V_h | 1].T @ E_h^T accumulated over k-tiles (den rides
      as the 65th matmul row, costing no extra PE time)
  O_norm = O * (1/den);  1/den = exp(-ln(den)) on ACT (custom-DVE
    reciprocal is broken on HW), broadcast across the 64 feat partitions
    via a DRAM-roundtrip DMA with a 0-stride AP.
  out_partial[tok, :] = O_norm^T.T @ WoT_slice  (bf16)
"""

import contextlib
import numpy as np
import ml_dtypes

B, N, D = 2, 2048, 1024
H, DH = 16, 64
NCORES = 8
CPB = 4                  # cores per batch
HPC = H // CPB           # heads per core
F = HPC * DH             # feature slice per core (256)
T = N                    # tokens per core (one batch)
NPAIR = HPC // 2         # head pairs per core
SCALE = DH ** -0.5
P = 128
KC = D // P              # k-chunks over the model dim
NT = T // 512            # 512-token chunks
QC = N // 512            # q chunks per batch
JT = N // P              # 128-token k tiles per batch

BF16 = ml_dtypes.bfloat16

_BUILT = {}


def _build_program(debug=False, loop=0):
    import concourse.bass as bass
    import concourse.tile as tile
    from concourse import mybir
    from concourse.bacc import Bacc

    f32 = mybir.dt.float32
    bf16 = mybir.dt.bfloat16
    EXP = mybir.ActivationFunctionType.Exp

    class BaccOneActTable(Bacc):
        """Force every activation onto the natural_log_exp_and_others table
        set (it contains Exp, Ln, Copy and Identity) so the ACT engine loads
        its function table exactly once instead of thrashing between the
        exp and ln sets (~2.7us per reload)."""

        def insert_act_table_loads(self):
            import bass_rust as _bass_rust
            from concourse.hw_specs import get_activation_tables

            has_activation = any(
                isinstance(i, mybir.InstActivation)
                for blk in self.main_func.blocks
                for i in blk.instructions
            )
            if not has_activation:
                return
            keep = "natural_log_exp_and_others"
            tables = [
                (nm, (fns if nm == keep else set()))
                for nm, fns in get_activation_tables(self.m.arch).items()
            ]
            _bass_rust.insert_act_table_loads(self, tables)

    nc = BaccOneActTable()
    xt = nc.declare_dram_parameter("xt", [D, T], bf16, isOutput=False)
    wq = nc.declare_dram_parameter("wq", [D, F], bf16, isOutput=False)
    wk = nc.declare_dram_parameter("wk", [D, F], bf16, isOutput=False)
    wv = nc.declare_dram_parameter("wv", [D, F], bf16, isOutput=False)
    wo = nc.declare_dram_parameter("wo", [F, D], bf16, isOutput=False)
    out = nc.declare_dram_parameter("out", [T, D], bf16, isOutput=True)
    dbg = {}
    if debug:
        for nm, shp, dt_ in [
            ("d_qt", [P, NPAIR, T], bf16), ("d_kt", [P, NPAIR, T], bf16),
            ("d_va", [P, JT * HPC * (DH + 1)], bf16),
            ("d_ot", [P, NPAIR, T], bf16), ("d_et", [P, 1024], bf16),
            ("d_st", [P, 1024], f32), ("d_ov", [P, 1024], f32),
        ]:
            dbg[nm] = nc.declare_dram_parameter(nm, shp, dt_, isOutput=True)

    with tile.TileContext(nc) as tc:
        with contextlib.ExitStack() as ctx:
            persist = ctx.enter_context(tc.tile_pool(name="persist", bufs=1))
            work = ctx.enter_context(tc.tile_pool(name="work", bufs=1))
            dpool = ctx.enter_context(
                tc.tile_pool(name="dscratch", bufs=1, space="DRAM"))

            # ---- persistent SBUF tensors ----
            xt_sb = persist.tile([P, KC, T], bf16)          # x^T, chunked over D
            wq_sb = persist.tile([P, KC, F], bf16)
            wk_sb = persist.tile([P, KC, F], bf16)
            wv_sb = persist.tile([P, KC, F], bf16)
            wo_sb = persist.tile([P, NPAIR, D], bf16)
            qt_sb = persist.tile([P, NPAIR, T], bf16)       # Q^T (scaled)
            kt_sb = persist.tile([P, NPAIR, T], bf16)       # K^T
            # V per 128-tok k-tile: 4 head slots of [V_h (64 cols) | ones]
            va_sb = persist.tile([P, JT, HPC, DH + 1], bf16)
            ot_sb = persist.tile([P, NPAIR, T], bf16)       # normalized O^T
            et_all = persist.tile([P, 3, 1024], bf16)       # E^T rotation bufs
            # causal keep-mask for the diagonal 128x128 block: 1 iff c >= p
            trimask = persist.tile([P, P], bf16)

            # ---- constants ----
            nc.gpsimd.memset(trimask, 1.0)
            nc.gpsimd.affine_select(
                out=trimask, in_=trimask,
                compare_op=mybir.AluOpType.is_ge,
                fill=0.0, base=0, pattern=[[1, P]], channel_multiplier=-1,
            )
            nc.gpsimd.memset(va_sb[:, :, :, DH], 1.0)
            nc.gpsimd.memset(et_all, 0.0)

            psum = tc.alloc_tile_pool(name="psum", bufs=1, space="PSUM")

            def body():
                # ---- load inputs (nt-major so projections start early) ----
                nc.sync.dma_start(out=wq_sb, in_=wq.rearrange("(a p) f -> p a f", p=P))
                nc.sync.dma_start(out=wk_sb, in_=wk.rearrange("(a p) f -> p a f", p=P))
                nc.sync.dma_start(out=wv_sb, in_=wv.rearrange("(a p) f -> p a f", p=P))
                xt_r = xt.rearrange("(a p) t -> p a t", p=P)
                for nt in range(NT):
                    nc.sync.dma_start(
                        out=xt_sb[:, :, nt * 512:(nt + 1) * 512],
                        in_=xt_r[:, :, nt * 512:(nt + 1) * 512])
                nc.sync.dma_start(out=wo_sb, in_=wo.rearrange("(a p) f -> p a f", p=P))

                # ---- projections: qT, kT (feat-major), V (tok-major) ----
                for nt in range(NT):
                    c0 = nt * 512
                    for p2 in range(NPAIR):
                        fsl = slice(p2 * P, (p2 + 1) * P)
                        pq = psum.tile([P, 1024], f32, tag="st", bufs=2,
                                       name=f"pq{nt}_{p2}")
                        for kc in range(KC):
                            nc.tensor.matmul(
                                pq[:, 0:512], wq_sb[:, kc, fsl],
                                xt_sb[:, kc, c0:c0 + 512],
                                start=(kc == 0), stop=(kc == KC - 1))
                        nc.vector.tensor_copy(qt_sb[:, p2, c0:c0 + 512],
                                              pq[:, 0:512])
                        pk = psum.tile([P, 1024], f32, tag="st", bufs=2,
                                       name=f"pk{nt}_{p2}")
                        for kc in range(KC):
                            nc.tensor.matmul(
                                pk[:, 0:512], wk_sb[:, kc, fsl],
                                xt_sb[:, kc, c0:c0 + 512],
                                start=(kc == 0), stop=(kc == KC - 1))
                        nc.vector.tensor_copy(kt_sb[:, p2, c0:c0 + 512],
                                              pk[:, 0:512])
                    for j4 in range(4):
                        tt = nt * 4 + j4
                        t0 = c0 + j4 * P
                        pv = psum.tile([P, HPC, DH], f32,
                                       tag=("ov0" if j4 % 2 == 0 else "ov1"),
                                       bufs=1, name=f"pv{tt}")
                        for kc in range(KC):
                            nc.tensor.matmul(
                                pv, xt_sb[:, kc, t0:t0 + P], wv_sb[:, kc, :],
                                start=(kc == 0), stop=(kc == KC - 1))
                        nc.vector.tensor_copy(va_sb[:, tt, :, 0:DH], pv)

                if debug:
                    nc.sync.dma_start(out=dbg["d_qt"][:], in_=qt_sb)
                    nc.sync.dma_start(out=dbg["d_kt"][:], in_=kt_sb)
                    nc.sync.dma_start(
                        out=dbg["d_va"][:],
                        in_=va_sb.rearrange("p a b c -> p (a b c)"))

                # ---- attention + output projection ----
                def emit_wo(qg0):
                    for ti in range(4):
                        t0 = qg0 + ti * P
                        wop = psum.tile([P, 1024], f32, tag="wop", bufs=1,
                                        name=f"wop{t0}")
                        for nn in range(2):
                            nsl = slice(nn * 512, (nn + 1) * 512)
                            nc.tensor.matmul(
                                wop[:, nsl], ot_sb[:, 0, t0:t0 + P],
                                wo_sb[:, 0, nsl], start=True, stop=False)
                            nc.tensor.matmul(
                                wop[:, nsl], ot_sb[:, 1, t0:t0 + P],
                                wo_sb[:, 1, nsl], start=False, stop=True)
                        wos = work.tile([P, 1024], bf16, tag="wos", bufs=2,
                                        name=f"wos{t0}")
                        nc.vector.tensor_copy(wos, wop)
                        nc.sync.dma_start(out=out[t0:t0 + P, :], in_=wos)

                et_idx = 0
                pending_wo = None
                for qc in range(QC):
                    qg = qc * 512
                    jmax = (qc + 1) * 4       # k tiles needed (causal)
                    for pr in range(NPAIR):
                        ovc = work.tile([DH + 1, 1024], f32, tag="ovc",
                                        bufs=2, name=f"ovc{qc}_{pr}")
                        ovs = []
                        for hh in range(2):
                            ov = psum.tile([P, 512], f32, tag=f"ov{hh}",
                                           bufs=1, name=f"ov{hh}_{qc}{pr}")
                            ovs.append(ov[0:DH + 1, :])  # [O(0:64); den(64)]
                        for j in range(jmax):
                            kg = j * P
                            # pfx: fully-masked q-column prefix of this tile
                            pfx = max(0, (j - qc * 4)) * P
                            st = psum.tile([P, 1024], f32, tag="st", bufs=2,
                                           name=f"st{qc}_{pr}_{j}")
                            et = et_all[:, et_idx % 3, :]
                            et_idx += 1
                            for hh in range(2):
                                hsl = slice(hh * DH, (hh + 1) * DH)
                                nc.tensor.matmul(
                                    st[:, hh * 512 + pfx:(hh + 1) * 512],
                                    kt_sb[hsl, pr, kg:kg + P],
                                    qt_sb[hsl, pr, qg + pfx:qg + 512],
                                    start=True, stop=True)
                            if pfx == 0:
                                nc.scalar.activation(et, st, EXP)
                            else:
                                for hh in range(2):
                                    esl = slice(hh * 512 + pfx, (hh + 1) * 512)
                                    nc.scalar.activation(et[:, esl], st[:, esl],
                                                         EXP)
                            if j >= qc * 4:   # diag: causal mask
                                for hh in range(2):
                                    blk = slice(hh * 512 + pfx,
                                                hh * 512 + pfx + P)
                                    nc.vector.tensor_mul(
                                        et[:, blk], et[:, blk], trimask)
                            if debug and qc == 0 and pr == 0 and j == 0:
                                std = work.tile([P, 1024], f32, tag="std",
                                                bufs=1)
                                nc.vector.tensor_copy(std, st)
                                nc.sync.dma_start(out=dbg["d_st"][:], in_=std)
                                nc.sync.dma_start(out=dbg["d_et"][:], in_=et)
                            for hh in range(2):
                                nc.tensor.matmul(
                                    ovs[hh][:, pfx:512],
                                    va_sb[:, j, 2 * pr + hh, :],
                                    et[:, hh * 512 + pfx:(hh + 1) * 512],
                                    start=(j == 0), stop=(j == jmax - 1))
                        # move [O; den] out of PSUM right away so the ov
                        # slots free for the next pair/q-chunk
                        nc.vector.tensor_copy(ovc[:, 0:512], ovs[0])
                        nc.vector.tensor_copy(ovc[:, 512:1024], ovs[1])
                        if debug and qc == 0 and pr == 0:
                            ovd = work.tile([P, 1024], f32, tag="ovd", bufs=1)
                            nc.vector.tensor_copy(ovd[0:DH + 1, :], ovc)
                            nc.sync.dma_start(out=dbg["d_ov"][:], in_=ovd)
                        # 1/den = exp(-ln(den)) on ACT, broadcast over 64
                        # partitions via a DRAM roundtrip
                        lse = work.tile([DH + 1, 1024], f32, tag="lse", bufs=2,
                                        name=f"lse{qc}_{pr}")
                        nc.scalar.activation(
                            lse[DH:DH + 1, :], ovc[DH:DH + 1, :],
                            mybir.ActivationFunctionType.Ln)
                        rden = work.tile([DH + 1, 1024], f32, tag="rden",
                                         bufs=2, name=f"rden{qc}_{pr}")
                        nc.scalar.activation(
                            rden[DH:DH + 1, :], lse[DH:DH + 1, :],
                            mybir.ActivationFunctionType.Exp, scale=-1.0)
                        dsc = dpool.tile([1, 1024], f32, tag="dsc", bufs=2,
                                         name=f"dsc{qc}_{pr}")
                        nc.sync.dma_start(out=dsc, in_=rden[DH:DH + 1, :])
                        rbc = work.tile([DH, 1024], f32, tag="rbc", bufs=2,
                                        name=f"rbc{qc}_{pr}")
                        bc_ap = bass.AP(
                            tensor=dsc.tensor, offset=dsc.offset,
                            ap=[[0, DH], list(dsc.ap[-1])])
                        nc.sync.dma_start(out=rbc, in_=bc_ap)
                        nc.vector.tensor_mul(
                            ot_sb[0:DH, pr, qg:qg + 512], ovc[0:DH, 0:512],
                            rbc[:, 0:512])
                        otb = work.tile([DH, 512], bf16, tag="otb", bufs=2,
                                        name=f"otb{qc}_{pr}")
                        nc.vector.tensor_mul(otb, ovc[0:DH, 512:1024],
                                             rbc[:, 512:1024])
                        nc.sync.dma_start(out=ot_sb[DH:P, pr, qg:qg + 512],
                                          in_=otb)
                    # output projection, emitted one qc late so the PE
                    # reaches these matmuls only after the normalization
                    # DMA chain of the producing qc has long finished
                    if pending_wo is not None:
                        emit_wo(pending_wo)
                    pending_wo = qg
                if pending_wo is not None:
                    emit_wo(pending_wo)
                if debug:
                    nc.sync.dma_start(out=dbg["d_ot"][:], in_=ot_sb)

            if loop:
                with tc.For_i(0, loop, 1):
                    body()
            else:
                body()
            psum.release()

    nc.finalize()
    return nc


def _get_program(debug=False):
    key = ("ncd" if debug else "nc")
    if key not in _BUILT:
        _BUILT[key] = _build_program(debug=debug)
    return _BUILT[key]


def _prep_inputs(x, Wq, Wkv, Wo):
    maps = []
    xts = [np.ascontiguousarray(x[b].T).astype(BF16) for b in range(B)]
    for c in range(NCORES):
        b, g = divmod(c, CPB)
        r0 = g * F
        maps.append({
            "xt": xts[b],
            "wq": np.ascontiguousarray(Wq[r0:r0 + F, :].T * SCALE).astype(BF16),
            "wk": np.ascontiguousarray(Wkv[r0:r0 + F, :].T).astype(BF16),
            "wv": np.ascontiguousarray(Wkv[D + r0:D + r0 + F, :].T).astype(BF16),
            "wo": np.ascontiguousarray(Wo[:, r0:r0 + F].T).astype(BF16),
        })
    return maps


def kernel(x, Wq, Wkv, Wo):
    from concourse.bass_utils import run_bass_kernel_spmd

    nc = _get_program()
    in_maps = _prep_inputs(np.asarray(x, np.float32), np.asarray(Wq, np.float32),
                           np.asarray(Wkv, np.float32), np.asarray(Wo, np.float32))
    res = run_bass_kernel_spmd(nc, in_maps, list(range(NCORES)))
    acc = np.zeros((B, N, D), np.float32)
    for c in range(NCORES):
        b = c // CPB
        acc[b] += res.results[c]["out"].astype(np.float32)
    return acc
